# revision 14
# baseline (speedup 1.0000x reference)
"""Sparse multi-head attention (per-head strided K/V subsampling) for trn2.

Problem (hardcoded):
  query/key/value: (2048, 8, 512) f32, attn_mask: (8, 2048) bool,
  proj_w: (512, 512), proj_b: (512,).
  Per head h (8 heads, head_dim 64) with stride ksz in [4,4,2,2,1,1,1,1]:
    scores = q_h @ k_h[::ksz].T * 0.125, masked softmax over subsampled keys,
    o_h = softmax @ v_h[::ksz].
  Reference then does a RAW reshape (B,T,D)->(T,B,D) per head before concat +
  out-projection.  That reshape is a pure row permutation of the flattened
  (B*T, 512) matrix, so computing per-(batch,head) attention in (t, d) layout,
  concatenating per batch, projecting, stacking batches, and reshaping
  (B*T, 512) -> (T, B, 512) reproduces it exactly.

Sharding: batch-parallel, one batch element per NeuronCore (8 cores).

Device/layout design (measured-on-HW rationale):
  - mask-gather on the host: masked keys contribute exactly zero, so only
    unmasked subsampled keys are shipped (~50%). Pad rows are all-zero
    INCLUDING the ones-column of the V-augmented matrix, so pads add 0 to
    both numerator and denominator (their exp(0)=1 hits zero V rows).
  - all matmul operands fp16 (f32r streams ~3x slower per row on real HW).
  - scores computed transposed (s on partitions, t free); V augmented with a
    ones column so one accumulating matmul produces both the unnormalized
    output (rows 0:64 of po) and the softmax denominator (row 64).
  - qT/kT are stored TWICE, with the 64-row halves swapped in the copy: the
    two score matmuls of a chunk then run on opposite PE row groups, so they
    execute concurrently (row tiling) and the second LDWEIGHTS overlaps the
    first matmul instead of serializing.
  - the heads of a pair share their keep-set, so their sub-128 tail lanes
    are PACKED into one chunk (head A at partitions 0:64, head B at 64:128,
    exploiting row+col PE tiling) - one ACTIVATE instead of two for the
    tails, per pair per t-half.
  - exp fused on ACT: ex = exp(0.125 * scores) in one [128, 1024] ACTIVATE
    per chunk/t-half (ACT is the pacing engine: (N+352)/1.2ns per inst).
  - normalization: po rows are copied to SBUF in two DVE ops (releases the
    PSUM accumulator ~1.5us after the last PV matmul), then
    reciprocal_approx_fast (custom DVE ops need partition-0 SBUF inputs) +
    gpsimd partition-broadcast + DVE multiply, all off the critical path.
  - out-projection with host-side proj_w.T; bias added via DVE tensor_add
    with a pre-broadcast bias tile during the PSUM->SBUF copy. Projection
    chunks for t-half 0 are injected mid-head into the long heads of
    t-half 1 (j=2 and j=6, far enough apart for the ACT backlog to recover).
  - dummy matmul bursts keep the PE HAM clock gate warm: once during the
    initial DMA wait, once while the final head's normalization chain runs
    (so the projection tail executes at 2.4 GHz).
"""

import numpy as np

import concourse.bass as bass
import concourse.tile as tile
from concourse import bacc, mybir
from concourse.bass_utils import run_bass_kernel_spmd

T = 2048
B = 8
E = 512
H = 8
D = 64
KS = [4, 4, 2, 2, 1, 1, 1, 1]
SCALE = 0.125
P = 128
THALF = 1024
F32 = mybir.dt.float32
F16 = mybir.dt.float16


def build_program(nf4, nf2, nf1, t4, t2, t1):
    # nfX: number of full 128-lane chunks per stride group; tX: group has a
    # packed <=64-lane tail chunk shared by the head pair
    N4, N2, N1 = max(nf4, 1) * P, max(nf2, 1) * P, max(nf1, 1) * P
    nc = bacc.Bacc("TRN2", target_bir_lowering=False, debug=False, num_devices=B)

    qT = nc.dram_tensor("qT", [2 * E, T], F16, kind="ExternalInput")
    k4T = nc.dram_tensor("k4T", [2 * P, N4], F16, kind="ExternalInput")
    k2T = nc.dram_tensor("k2T", [2 * P, N2], F16, kind="ExternalInput")
    k1Ta = nc.dram_tensor("k1Ta", [2 * P, N1], F16, kind="ExternalInput")
    k1Tb = nc.dram_tensor("k1Tb", [2 * P, N1], F16, kind="ExternalInput")
    va4 = nc.dram_tensor("va4", [P, max(nf4, 1) * 130], F16,
                         kind="ExternalInput")
    va2 = nc.dram_tensor("va2", [P, max(nf2, 1) * 130], F16,
                         kind="ExternalInput")
    va1 = nc.dram_tensor("va1", [P, max(nf1, 1) * 260], F16,
                         kind="ExternalInput")
    kt4 = nc.dram_tensor("kt4", [2 * P, 64], F16, kind="ExternalInput")
    kt2 = nc.dram_tensor("kt2", [2 * P, 64], F16, kind="ExternalInput")
    kt1 = nc.dram_tensor("kt1", [2 * P, 128], F16, kind="ExternalInput")
    vat4 = nc.dram_tensor("vat4", [P, 130], F16, kind="ExternalInput")
    vat2 = nc.dram_tensor("vat2", [P, 130], F16, kind="ExternalInput")
    vat1 = nc.dram_tensor("vat1", [P, 260], F16, kind="ExternalInput")
    wT = nc.dram_tensor("wT", [E, E], F16, kind="ExternalInput")
    pb = nc.dram_tensor("pb", [1, E], F32, kind="ExternalInput")
    out = nc.dram_tensor("out", [T, E], F32, kind="ExternalOutput")

    NCHF = [nf4, nf4, nf2, nf2, nf1, nf1, nf1, nf1]
    TAIL = [t4, t4, t2, t2, t1, t1, t1, t1]

    with tile.TileContext(nc) as tc:
        with (
            tc.tile_pool(name="const", bufs=1) as cpool,
            tc.tile_pool(name="exp", bufs=4) as epool,
            tc.tile_pool(name="norm", bufs=2) as npool,
            tc.tile_pool(name="outsb", bufs=4) as opool,
            tc.tile_pool(name="psA", bufs=1, space="PSUM") as pspool,
        ):
            # ---- persistent SBUF loads (ordered by first use) ----
            qTA_sb, qTB_sb = [], []
            for p_ in range(4):
                qTA_sb.append(cpool.tile([P, T], F16, name=f"qTA{p_}",
                                         tag=f"qTA{p_}"))
                qTB_sb.append(cpool.tile([P, T], F16, name=f"qTB{p_}",
                                         tag=f"qTB{p_}"))
            nc.sync.dma_start(qTA_sb[0][:], qT.ap()[0:P, :])
            nc.sync.dma_start(qTB_sb[0][:], qT.ap()[E:E + P, :])

            def kpair(name, dram, N):
                a = cpool.tile([P, N], F16, name=name + "a", tag=name + "a")
                b = cpool.tile([P, N], F16, name=name + "b", tag=name + "b")
                nc.sync.dma_start(a[:], dram.ap()[0:P, :])
                nc.sync.dma_start(b[:], dram.ap()[P:2 * P, :])
                return a, b

            k4A, k4B = kpair("k4", k4T, N4)
            va4_sb = cpool.tile([P, max(nf4, 1) * 130], F16, name="va4s",
                                tag="va4s")
            nc.sync.dma_start(va4_sb[:], va4.ap())
            kt4A, kt4B = kpair("kt4", kt4, 64)
            vat4_sb = cpool.tile([P, 130], F16, name="vat4s", tag="vat4s")
            nc.sync.dma_start(vat4_sb[:], vat4.ap())
            k2A, k2B = kpair("k2", k2T, N2)
            nc.sync.dma_start(qTA_sb[1][:], qT.ap()[P:2 * P, :])
            nc.sync.dma_start(qTB_sb[1][:], qT.ap()[E + P:E + 2 * P, :])
            va2_sb = cpool.tile([P, max(nf2, 1) * 130], F16, name="va2s",
                                tag="va2s")
            nc.sync.dma_start(va2_sb[:], va2.ap())
            kt2A, kt2B = kpair("kt2", kt2, 64)
            vat2_sb = cpool.tile([P, 130], F16, name="vat2s", tag="vat2s")
            nc.sync.dma_start(vat2_sb[:], vat2.ap())
            k1aA, k1aB = kpair("k1a", k1Ta, N1)
            nc.sync.dma_start(qTA_sb[2][:], qT.ap()[2 * P:3 * P, :])
            nc.sync.dma_start(qTB_sb[2][:], qT.ap()[E + 2 * P:E + 3 * P, :])
            k1bA, k1bB = kpair("k1b", k1Tb, N1)
            nc.sync.dma_start(qTA_sb[3][:], qT.ap()[3 * P:4 * P, :])
            nc.sync.dma_start(qTB_sb[3][:], qT.ap()[E + 3 * P:E + 4 * P, :])
            va1_sb = cpool.tile([P, max(nf1, 1) * 260], F16, name="va1s",
                                tag="va1s")
            nc.sync.dma_start(va1_sb[:], va1.ap())
            kt1A, kt1B = kpair("kt1", kt1, 128)
            vat1_sb = cpool.tile([P, 260], F16, name="vat1s", tag="vat1s")
            nc.sync.dma_start(vat1_sb[:], vat1.ap())
            wT_sb = []
            for i in range(4):
                t_ = cpool.tile([P, E], F16, name=f"wT{i}", tag=f"wT{i}")
                nc.sync.dma_start(t_[:], wT.ap()[i * P:(i + 1) * P, :])
                wT_sb.append(t_)
            pb_sb = cpool.tile([1, E], F32, name="pbs", tag="pbs")
            nc.sync.dma_start(pb_sb[:], pb.ap())
            pbb_sb = cpool.tile([P, E], F32, name="pbb", tag="pbb")
            nc.gpsimd.partition_broadcast(pbb_sb[:], pb_sb[:])

            # ---- PE warm-up burst ----
            wu_sb = cpool.tile([64, 512], F16, name="wu", tag="wu")
            nc.gpsimd.memset(wu_sb[:], 0.0)

            def warm_burst(n):
                wps = pspool.tile([P, THALF], F32, name="ps", tag="ps",
                                  bufs=2)
                for _ in range(n):
                    nc.tensor.matmul(
                        wps[0:4, 0:512], lhsT=wu_sb[:, 0:4], rhs=wu_sb[:],
                        start=True, stop=True)

            warm_burst(8)

            # per-head views: (tile, row0) for tq0 (natural) / tq1 (swapped)
            def kT_h(h, tq):
                A, Bt = [(k4A, k4B), (k4A, k4B), (k2A, k2B), (k2A, k2B),
                         (k1aA, k1aB), (k1aA, k1aB), (k1bA, k1bB),
                         (k1bA, k1bB)][h]
                if tq == 0:
                    return A, (h % 2) * 64
                return Bt, (1 - h % 2) * 64

            def qT_h(h, tq):
                if tq == 0:
                    return qTA_sb[h // 2], (h % 2) * 64
                return qTB_sb[h // 2], (1 - h % 2) * 64

            def va_h(h, j):
                if h < 2:
                    return va4_sb[:, j * 130 + h * 65: j * 130 + h * 65 + 65]
                if h < 4:
                    return va2_sb[:, j * 130 + (h - 2) * 65:
                                  j * 130 + (h - 2) * 65 + 65]
                return va1_sb[:, j * 260 + (h - 4) * 65:
                              j * 260 + (h - 4) * 65 + 65]

            def kt_pair(pr):
                # (Atile, Btile, col0) for the packed tail of head pair pr
                if pr == 0:
                    return kt4A, kt4B, 0
                if pr == 1:
                    return kt2A, kt2B, 0
                if pr == 2:
                    return kt1A, kt1B, 0
                return kt1A, kt1B, 64

            def vat_pair(pr):
                # (tile, col0): head even at rows 0:64 cols c0:c0+65,
                # head odd at rows 64:128 cols c0+65:c0+130
                if pr == 0:
                    return vat4_sb, 0
                if pr == 1:
                    return vat2_sb, 0
                if pr == 2:
                    return vat1_sb, 0
                return vat1_sb, 130

            # transposed normalized head outputs (fp16), feeding proj
            oT_sb = []
            for p_ in range(4):
                t_ = cpool.tile([P, T], F16, name=f"oT{p_}", tag=f"oT{p_}")
                oT_sb.append(t_)

            def proj_chunk(tq):
                pp_full = pspool.tile([P, THALF], F32, name="pp", tag="ps",
                                      bufs=2)
                pp = pp_full[:, 0:E]
                for i in range(4):
                    nc.tensor.matmul(
                        pp, lhsT=oT_sb[i][:, tq * P:(tq + 1) * P],
                        rhs=wT_sb[i][:], start=(i == 0), stop=(i == 3))
                ot = opool.tile([P, E], F32, name="ot", tag="ot")
                nc.vector.tensor_add(ot[:], pp, pbb_sb[:])
                nc.sync.dma_start(out.ap()[tq * P:(tq + 1) * P, :], ot[:])

            def norm(h, po_, t0):
                # two DVE copies release po quickly; recip/broadcast/multiply
                # run from SBUF off the fast path. den must be a partition-0
                # tile: custom DVE ops misread non-zero base partitions.
                oU = npool.tile([64, THALF], F32, name="oU", tag="oU")
                nc.vector.tensor_copy(oU[:], po_[0:64, :])
                den = npool.tile([1, THALF], F32, name="den", tag="den")
                nc.vector.tensor_copy(den[:], po_[64:65, :])
                rec = npool.tile([1, THALF], F32, name="rec", tag="rec")
                nc.vector.reciprocal_approx_fast(rec[:], den[:])
                rbc = npool.tile([64, THALF], F32, name="rbc", tag="rbc")
                nc.gpsimd.partition_broadcast(rbc[:], rec[:])
                r0 = (h % 2) * 64
                nc.vector.tensor_mul(
                    oT_sb[h // 2][r0:r0 + 64, t0:t0 + THALF], oU[:], rbc[:])

            # ---- attention main loop ----
            for th in range(2):
                t0 = th * THALF
                for pr in range(4):
                    h0, h1 = 2 * pr, 2 * pr + 1
                    nf = NCHF[h0]
                    hastail = TAIL[h0]
                    po0 = pspool.tile([P, THALF], F32, name="po", tag="po",
                                      bufs=2)
                    po1 = pspool.tile([P, THALF], F32, name="po", tag="po",
                                      bufs=2)

                    def pv(h, po_, exs_, j, last):
                        for tq in range(2):
                            nc.tensor.matmul(
                                po_[0:65, tq * 512:(tq + 1) * 512],
                                lhsT=va_h(h, j),
                                rhs=exs_[j][:, tq * 512:(tq + 1) * 512],
                                start=(j == 0), stop=last)

                    def full_head(h, po_):
                        exs = []
                        for j in range(nf):
                            ps = pspool.tile([P, THALF], F32, name="ps",
                                             tag="ps", bufs=2)
                            for tq in range(2):
                                kt, kr = kT_h(h, tq)
                                qt, qr = qT_h(h, tq)
                                nc.tensor.matmul(
                                    ps[:, tq * 512:(tq + 1) * 512],
                                    lhsT=kt[kr:kr + 64, j * P:(j + 1) * P],
                                    rhs=qt[qr:qr + 64, t0 + tq * 512:
                                           t0 + (tq + 1) * 512],
                                    start=True, stop=True)
                            ex = epool.tile([P, THALF], F16, name="ex",
                                            tag="ex", bufs=6)
                            nc.scalar.activation(
                                ex[:], ps[:],
                                mybir.ActivationFunctionType.Exp,
                                bias=0.0, scale=SCALE)
                            exs.append(ex)
                            if j >= 1:
                                pv(h, po_, exs, j - 1, last=False)
                            # inject t-half-0 projection chunks into the
                            # long heads of t-half 1; j=2/6 are far enough
                            # apart for the ACT backlog to recover
                            if th == 1 and h >= 4 and j in (2, 6):
                                proj_chunk(2 * (h - 4) + (0 if j == 2 else 1))
                        if nf >= 1:
                            pv(h, po_, exs, nf - 1, last=not hastail)

                    full_head(h0, po0)
                    full_head(h1, po1)

                    if hastail:
                        # packed tail: head-even lanes at partitions 0:64,
                        # head-odd at 64:128; 4 score MMs on 4 distinct PE
                        # quadrants run concurrently, one exp covers both
                        ktA, ktB, kc0 = kt_pair(pr)
                        ps = pspool.tile([P, THALF], F32, name="ps",
                                         tag="ps", bufs=2)
                        for hh, rbase in ((h0, 0), (h1, 64)):
                            for tq in range(2):
                                if tq == 0:
                                    kt, kr = ktA, (hh % 2) * 64
                                else:
                                    kt, kr = ktB, (1 - hh % 2) * 64
                                qt, qr = qT_h(hh, tq)
                                nc.tensor.matmul(
                                    ps[rbase:rbase + 64,
                                       tq * 512:(tq + 1) * 512],
                                    lhsT=kt[kr:kr + 64, kc0:kc0 + 64],
                                    rhs=qt[qr:qr + 64, t0 + tq * 512:
                                           t0 + (tq + 1) * 512],
                                    start=True, stop=True)
                        ext = epool.tile([P, THALF], F16, name="ex",
                                         tag="ex", bufs=6)
                        nc.scalar.activation(
                            ext[:], ps[:], mybir.ActivationFunctionType.Exp,
                            bias=0.0, scale=SCALE)
                        vt, vc0 = vat_pair(pr)
                        for po_, rbase, vc in ((po0, 0, vc0),
                                               (po1, 64, vc0 + 65)):
                            for tq in range(2):
                                nc.tensor.matmul(
                                    po_[0:65, tq * 512:(tq + 1) * 512],
                                    lhsT=vt[rbase:rbase + 64, vc:vc + 65],
                                    rhs=ext[rbase:rbase + 64,
                                            tq * 512:(tq + 1) * 512],
                                    start=(nf == 0), stop=True)

                    # bridge the final normalization chain with dummy
                    # matmuls so the PE stays warm into the projection tail
                    if th == 1 and pr == 3:
                        norm(h0, po0, t0)
                        norm(h1, po1, t0)
                        warm_burst(14)
                    else:
                        norm(h0, po0, t0)
                        norm(h1, po1, t0)
            for tq in range(8, 16):
                proj_chunk(tq)

    nc.compile()
    return nc


_PROGRAMS = {}


def _get_program(key):
    if key not in _PROGRAMS:
        _PROGRAMS[key] = build_program(*key)
    return _PROGRAMS[key]


def _swap_halves(m):
    # [128k, N] -> swap the two 64-row halves within each 128-row block
    blocks = [m[i:i + P] for i in range(0, m.shape[0], P)]
    return np.vstack([np.vstack([b[64:P], b[0:64]]) for b in blocks])


def _prep_core_inputs(query, key, value, wT, pb, keeps, cfg):
    nf4, nf2, nf1, t4, t2, t1 = cfg
    NF = {4: max(nf4, 1), 2: max(nf2, 1), 1: max(nf1, 1)}
    NFR = {4: nf4, 2: nf2, 1: nf1}
    ins = []
    for b in range(B):
        qb = np.ascontiguousarray(query[:, b, :].T).astype(np.float16)
        qbd = np.vstack([qb, _swap_halves(qb)])

        def build_k(sub, idx, c0, c1, ks):
            z = np.zeros((P, NF[ks] * P), dtype=np.float16)
            g = sub[idx[:NFR[ks] * P]]
            n = g.shape[0]
            z[:, 0:n] = g[:, c0:c1].T.astype(np.float16)
            return np.vstack([z, _swap_halves(z)])

        def build_va(sub, idx, heads, W, ks):
            g = sub[idx[:NFR[ks] * P]]
            z = np.zeros((P, NF[ks] * W), dtype=np.float16)
            for j in range(NF[ks]):
                seg = g[j * P:(j + 1) * P]
                m = seg.shape[0]
                if m == 0:
                    break
                for i, h in enumerate(heads):
                    z[0:m, j * W + i * 65: j * W + i * 65 + 64] = \
                        seg[:, h * 64:(h + 1) * 64].astype(np.float16)
                    z[0:m, j * W + i * 65 + 64] = 1.0
            return z

        def build_kt(sub, idx, pairs, ks):
            # [2P, 64*len(pairs)]: per pair, head-even dims on rows 0:64,
            # head-odd on 64:128, tail lanes as columns; plus swapped copy
            z = np.zeros((P, 64 * len(pairs)), dtype=np.float16)
            g = sub[idx[NFR[ks] * P:]]
            n = g.shape[0]
            for i, (ha, hb) in enumerate(pairs):
                if n:
                    z[0:64, i * 64:i * 64 + n] = \
                        g[:, ha * 64:(ha + 1) * 64].T.astype(np.float16)
                    z[64:P, i * 64:i * 64 + n] = \
                        g[:, hb * 64:(hb + 1) * 64].T.astype(np.float16)
            return np.vstack([z, _swap_halves(z)])

        def build_vat(sub, idx, pairs, ks):
            z = np.zeros((P, 130 * len(pairs)), dtype=np.float16)
            g = sub[idx[NFR[ks] * P:]]
            n = g.shape[0]
            for i, (ha, hb) in enumerate(pairs):
                if n:
                    z[0:n, i * 130:i * 130 + 64] = \
                        g[:, ha * 64:(ha + 1) * 64].astype(np.float16)
                    z[0:n, i * 130 + 64] = 1.0
                    z[64:64 + n, i * 130 + 65:i * 130 + 129] = \
                        g[:, hb * 64:(hb + 1) * 64].astype(np.float16)
                    z[64:64 + n, i * 130 + 129] = 1.0
            return z

        kb, vb = key[:, b, :], value[:, b, :]
        i4, i2, i1 = keeps[4][b], keeps[2][b], keeps[1][b]
        ins.append({
            "qT": qbd,
            "k4T": build_k(kb[::4], i4, 0, 128, 4),
            "k2T": build_k(kb[::2], i2, 128, 256, 2),
            "k1Ta": build_k(kb, i1, 256, 384, 1),
            "k1Tb": build_k(kb, i1, 384, 512, 1),
            "va4": build_va(vb[::4], i4, [0, 1], 130, 4),
            "va2": build_va(vb[::2], i2, [2, 3], 130, 2),
            "va1": build_va(vb, i1, [4, 5, 6, 7], 260, 1),
            "kt4": build_kt(kb[::4], i4, [(0, 1)], 4),
            "kt2": build_kt(kb[::2], i2, [(2, 3)], 2),
            "kt1": build_kt(kb, i1, [(4, 5), (6, 7)], 1),
            "vat4": build_vat(vb[::4], i4, [(0, 1)], 4),
            "vat2": build_vat(vb[::2], i2, [(2, 3)], 2),
            "vat1": build_vat(vb, i1, [(4, 5), (6, 7)], 1),
            "wT": wT, "pb": pb,
        })
    return ins


def kernel(query, key, value, attn_mask, proj_w, proj_b, _trace=False,
           **run_kwargs):
    query = np.asarray(query, dtype=np.float32)
    key = np.asarray(key, dtype=np.float32)
    value = np.asarray(value, dtype=np.float32)
    mask = np.asarray(attn_mask).astype(bool)
    wT = np.ascontiguousarray(
        np.asarray(proj_w, dtype=np.float32).T).astype(np.float16)
    pb = np.ascontiguousarray(
        np.asarray(proj_b, dtype=np.float32).reshape(1, E))

    keeps = {ks: [np.flatnonzero(~mask[b, ::ks]) for b in range(B)]
             for ks in (4, 2, 1)}
    cfg = []
    for ks in (4, 2, 1):
        mx = max(len(keeps[ks][b]) for b in range(B))
        nf, lt = divmod(mx, P)
        if lt > 64:
            # tail too wide to pack two heads side by side: pad to a full
            # chunk instead
            nf, lt = nf + 1, 0
        cfg.append((nf, lt > 0))
    cfg = (cfg[0][0], cfg[1][0], cfg[2][0], cfg[0][1], cfg[1][1], cfg[2][1])

    nc = _get_program(cfg)
    ins = _prep_core_inputs(query, key, value, wT, pb, keeps, cfg)
    res = run_bass_kernel_spmd(nc, ins, list(range(B)), trace=_trace,
                               **run_kwargs)
    outs = [np.asarray(res.results[b]["out"]) for b in range(B)]
    full = np.concatenate(outs, axis=0)          # (B*T, E), b-major rows
    result = full.reshape(T, B, E)
    if _trace:
        return result, res
    return result


# revision 16
# speedup vs baseline: 1.0916x; 1.0916x over previous
"""Sparse multi-head attention (per-head strided K/V subsampling) for trn2.

Problem (hardcoded):
  query/key/value: (2048, 8, 512) f32, attn_mask: (8, 2048) bool,
  proj_w: (512, 512), proj_b: (512,).
  Per head h (8 heads, head_dim 64) with stride ksz in [4,4,2,2,1,1,1,1]:
    scores = q_h @ k_h[::ksz].T * 0.125, masked softmax over subsampled keys,
    o_h = softmax @ v_h[::ksz].
  Reference then does a RAW reshape (B,T,D)->(T,B,D) per head before concat +
  out-projection.  That reshape is a pure row permutation of the flattened
  (B*T, 512) matrix, so computing per-(batch,head) attention in (t, d) layout,
  concatenating per batch, projecting, stacking batches, and reshaping
  (B*T, 512) -> (T, B, 512) reproduces it exactly.

Sharding: batch-parallel, one batch element per NeuronCore (8 cores).

Device/layout design (measured-on-HW rationale):
  - mask-gather on the host: masked keys contribute exactly zero, so only
    unmasked subsampled keys are shipped (~50%). Pad rows are all-zero
    INCLUDING the ones-column of the V-augmented matrix, so pads add 0 to
    both numerator and denominator (their exp(0)=1 hits zero V rows).
  - all matmul operands fp16 (f32r streams ~3x slower per row on real HW).
  - scores computed transposed (s on partitions, t free); V augmented with a
    ones column so one accumulating matmul produces both the unnormalized
    output (rows 0:64 of po) and the softmax denominator (row 64).
  - qT/kT are stored TWICE, with the 64-row halves swapped in the copy: the
    two score matmuls of a chunk then run on opposite PE row groups, so they
    execute concurrently (row tiling) and the second LDWEIGHTS overlaps the
    first matmul instead of serializing.
  - the heads of a pair share their keep-set, so their sub-128 tail lanes
    are PACKED into one chunk (head A at partitions 0:64, head B at 64:128,
    exploiting row+col PE tiling) - one ACTIVATE instead of two for the
    tails, per pair per t-half.
  - exp fused on ACT: ex = exp(0.125 * scores) in one [128, 1024] ACTIVATE
    per chunk/t-half (ACT is the pacing engine: (N+352)/1.2ns per inst).
  - normalization: po rows are copied to SBUF in two DVE ops (releases the
    PSUM accumulator ~1.5us after the last PV matmul), then
    reciprocal_approx_fast (custom DVE ops need partition-0 SBUF inputs) +
    gpsimd partition-broadcast + DVE multiply, all off the critical path.
  - out-projection with host-side proj_w.T; bias added via DVE tensor_add
    with a pre-broadcast bias tile during the PSUM->SBUF copy. Projection
    chunks for t-half 0 are injected mid-head into the long heads of
    t-half 1 (j=2 and j=6, far enough apart for the ACT backlog to recover).
  - dummy matmul bursts keep the PE HAM clock gate warm: once during the
    initial DMA wait, once while the final head's normalization chain runs
    (so the projection tail executes at 2.4 GHz).
"""

import numpy as np

import concourse.bass as bass
import concourse.tile as tile
from concourse import bacc, mybir
from concourse.bass_utils import run_bass_kernel_spmd

T = 2048
B = 8
E = 512
H = 8
D = 64
KS = [4, 4, 2, 2, 1, 1, 1, 1]
SCALE = 0.125
P = 128
THALF = 1024
F32 = mybir.dt.float32
F16 = mybir.dt.float16


def build_program(nf4, nf2, nf1, t4, t2, t1):
    # nfX: number of full 128-lane chunks per stride group; tX: group has a
    # packed <=64-lane tail chunk shared by the head pair
    N4, N2, N1 = max(nf4, 1) * P, max(nf2, 1) * P, max(nf1, 1) * P
    nc = bacc.Bacc("TRN2", target_bir_lowering=False, debug=False, num_devices=B)

    qT = nc.dram_tensor("qT", [2 * E, T], F16, kind="ExternalInput")
    k4T = nc.dram_tensor("k4T", [2 * P, N4], F16, kind="ExternalInput")
    k2T = nc.dram_tensor("k2T", [2 * P, N2], F16, kind="ExternalInput")
    k1Ta = nc.dram_tensor("k1Ta", [2 * P, N1], F16, kind="ExternalInput")
    k1Tb = nc.dram_tensor("k1Tb", [2 * P, N1], F16, kind="ExternalInput")
    va4 = nc.dram_tensor("va4", [P, max(nf4, 1) * 130], F16,
                         kind="ExternalInput")
    va2 = nc.dram_tensor("va2", [P, max(nf2, 1) * 130], F16,
                         kind="ExternalInput")
    va1 = nc.dram_tensor("va1", [P, max(nf1, 1) * 260], F16,
                         kind="ExternalInput")
    kt4 = nc.dram_tensor("kt4", [2 * P, 64], F16, kind="ExternalInput")
    kt2 = nc.dram_tensor("kt2", [2 * P, 64], F16, kind="ExternalInput")
    kt1 = nc.dram_tensor("kt1", [2 * P, 128], F16, kind="ExternalInput")
    vat4 = nc.dram_tensor("vat4", [P, 130], F16, kind="ExternalInput")
    vat2 = nc.dram_tensor("vat2", [P, 130], F16, kind="ExternalInput")
    vat1 = nc.dram_tensor("vat1", [P, 260], F16, kind="ExternalInput")
    wT = nc.dram_tensor("wT", [E, E], F16, kind="ExternalInput")
    pb = nc.dram_tensor("pb", [1, E], F32, kind="ExternalInput")
    out = nc.dram_tensor("out", [T, E], F32, kind="ExternalOutput")

    NCHF = [nf4, nf4, nf2, nf2, nf1, nf1, nf1, nf1]
    TAIL = [t4, t4, t2, t2, t1, t1, t1, t1]

    with tile.TileContext(nc) as tc:
        with (
            tc.tile_pool(name="const", bufs=1) as cpool,
            tc.tile_pool(name="exp", bufs=4) as epool,
            tc.tile_pool(name="norm", bufs=2) as npool,
            tc.tile_pool(name="outsb", bufs=4) as opool,
            tc.tile_pool(name="psA", bufs=1, space="PSUM") as pspool,
        ):
            # ---- persistent SBUF loads (ordered by first use) ----
            qTA_sb, qTB_sb = [], []
            for p_ in range(4):
                qTA_sb.append(cpool.tile([P, T], F16, name=f"qTA{p_}",
                                         tag=f"qTA{p_}"))
                qTB_sb.append(cpool.tile([P, T], F16, name=f"qTB{p_}",
                                         tag=f"qTB{p_}"))
            nc.sync.dma_start(qTA_sb[0][:], qT.ap()[0:P, :])
            nc.sync.dma_start(qTB_sb[0][:], qT.ap()[E:E + P, :])

            def kpair(name, dram, N):
                a = cpool.tile([P, N], F16, name=name + "a", tag=name + "a")
                b = cpool.tile([P, N], F16, name=name + "b", tag=name + "b")
                nc.sync.dma_start(a[:], dram.ap()[0:P, :])
                nc.sync.dma_start(b[:], dram.ap()[P:2 * P, :])
                return a, b

            kt4A, kt4B = kpair("kt4", kt4, 64)
            vat4_sb = cpool.tile([P, 130], F16, name="vat4s", tag="vat4s")
            nc.sync.dma_start(vat4_sb[:], vat4.ap())
            k4A, k4B = kpair("k4", k4T, N4)
            va4_sb = cpool.tile([P, max(nf4, 1) * 130], F16, name="va4s",
                                tag="va4s")
            nc.sync.dma_start(va4_sb[:], va4.ap())
            k2A, k2B = kpair("k2", k2T, N2)
            nc.sync.dma_start(qTA_sb[1][:], qT.ap()[P:2 * P, :])
            nc.sync.dma_start(qTB_sb[1][:], qT.ap()[E + P:E + 2 * P, :])
            va2_sb = cpool.tile([P, max(nf2, 1) * 130], F16, name="va2s",
                                tag="va2s")
            nc.sync.dma_start(va2_sb[:], va2.ap())
            kt2A, kt2B = kpair("kt2", kt2, 64)
            vat2_sb = cpool.tile([P, 130], F16, name="vat2s", tag="vat2s")
            nc.sync.dma_start(vat2_sb[:], vat2.ap())
            k1aA, k1aB = kpair("k1a", k1Ta, N1)
            nc.sync.dma_start(qTA_sb[2][:], qT.ap()[2 * P:3 * P, :])
            nc.sync.dma_start(qTB_sb[2][:], qT.ap()[E + 2 * P:E + 3 * P, :])
            k1bA, k1bB = kpair("k1b", k1Tb, N1)
            nc.sync.dma_start(qTA_sb[3][:], qT.ap()[3 * P:4 * P, :])
            nc.sync.dma_start(qTB_sb[3][:], qT.ap()[E + 3 * P:E + 4 * P, :])
            va1_sb = cpool.tile([P, max(nf1, 1) * 260], F16, name="va1s",
                                tag="va1s")
            nc.sync.dma_start(va1_sb[:], va1.ap())
            kt1A, kt1B = kpair("kt1", kt1, 128)
            vat1_sb = cpool.tile([P, 260], F16, name="vat1s", tag="vat1s")
            nc.sync.dma_start(vat1_sb[:], vat1.ap())
            wT_sb = []
            for i in range(4):
                t_ = cpool.tile([P, E], F16, name=f"wT{i}", tag=f"wT{i}")
                nc.sync.dma_start(t_[:], wT.ap()[i * P:(i + 1) * P, :])
                wT_sb.append(t_)
            pb_sb = cpool.tile([1, E], F32, name="pbs", tag="pbs")
            nc.sync.dma_start(pb_sb[:], pb.ap())
            pbb_sb = cpool.tile([P, E], F32, name="pbb", tag="pbb")
            nc.gpsimd.partition_broadcast(pbb_sb[:], pb_sb[:])

            # ---- PE warm-up burst ----
            wu_sb = cpool.tile([64, 512], F16, name="wu", tag="wu")
            nc.gpsimd.memset(wu_sb[:], 0.0)

            def warm_burst(n):
                wps = pspool.tile([P, THALF], F32, name="ps", tag="ps",
                                  bufs=2)
                for _ in range(n):
                    nc.tensor.matmul(
                        wps[0:4, 0:512], lhsT=wu_sb[:, 0:4], rhs=wu_sb[:],
                        start=True, stop=True)

            warm_burst(8)

            # per-head views: (tile, row0) for tq0 (natural) / tq1 (swapped)
            def kT_h(h, tq):
                A, Bt = [(k4A, k4B), (k4A, k4B), (k2A, k2B), (k2A, k2B),
                         (k1aA, k1aB), (k1aA, k1aB), (k1bA, k1bB),
                         (k1bA, k1bB)][h]
                if tq == 0:
                    return A, (h % 2) * 64
                return Bt, (1 - h % 2) * 64

            def qT_h(h, tq):
                if tq == 0:
                    return qTA_sb[h // 2], (h % 2) * 64
                return qTB_sb[h // 2], (1 - h % 2) * 64

            def va_h(h, j):
                if h < 2:
                    return va4_sb[:, j * 130 + h * 65: j * 130 + h * 65 + 65]
                if h < 4:
                    return va2_sb[:, j * 130 + (h - 2) * 65:
                                  j * 130 + (h - 2) * 65 + 65]
                return va1_sb[:, j * 260 + (h - 4) * 65:
                              j * 260 + (h - 4) * 65 + 65]

            def kt_pair(pr):
                # (Atile, Btile, col0) for the packed tail of head pair pr
                if pr == 0:
                    return kt4A, kt4B, 0
                if pr == 1:
                    return kt2A, kt2B, 0
                if pr == 2:
                    return kt1A, kt1B, 0
                return kt1A, kt1B, 64

            def vat_pair(pr):
                # (tile, col0): head even at rows 0:64 cols c0:c0+65,
                # head odd at rows 64:128 cols c0+65:c0+130
                if pr == 0:
                    return vat4_sb, 0
                if pr == 1:
                    return vat2_sb, 0
                if pr == 2:
                    return vat1_sb, 0
                return vat1_sb, 130

            # transposed normalized head outputs (fp16), feeding proj
            oT_sb = []
            for p_ in range(4):
                t_ = cpool.tile([P, T], F16, name=f"oT{p_}", tag=f"oT{p_}")
                oT_sb.append(t_)

            def proj_chunk(tq):
                pp_full = pspool.tile([P, THALF], F32, name="pp", tag="ps",
                                      bufs=2)
                pp = pp_full[:, 0:E]
                for i in range(4):
                    nc.tensor.matmul(
                        pp, lhsT=oT_sb[i][:, tq * P:(tq + 1) * P],
                        rhs=wT_sb[i][:], start=(i == 0), stop=(i == 3))
                ot = opool.tile([P, E], F32, name="ot", tag="ot")
                nc.vector.tensor_add(ot[:], pp, pbb_sb[:])
                nc.sync.dma_start(out.ap()[tq * P:(tq + 1) * P, :], ot[:])

            def norm(h, po_, t0):
                # two DVE copies release po quickly; recip/broadcast/multiply
                # run from SBUF off the fast path. den must be a partition-0
                # tile: custom DVE ops misread non-zero base partitions.
                oU = npool.tile([64, THALF], F32, name="oU", tag="oU")
                nc.vector.tensor_copy(oU[:], po_[0:64, :])
                den = npool.tile([1, THALF], F32, name="den", tag="den")
                nc.vector.tensor_copy(den[:], po_[64:65, :])
                rec = npool.tile([1, THALF], F32, name="rec", tag="rec")
                nc.vector.reciprocal_approx_fast(rec[:], den[:])
                rbc = npool.tile([64, THALF], F32, name="rbc", tag="rbc")
                nc.gpsimd.partition_broadcast(rbc[:], rec[:])
                r0 = (h % 2) * 64
                nc.vector.tensor_mul(
                    oT_sb[h // 2][r0:r0 + 64, t0:t0 + THALF], oU[:], rbc[:])

            # ---- attention main loop ----
            for th in range(2):
                t0 = th * THALF
                for pr in range(4):
                    h0, h1 = 2 * pr, 2 * pr + 1
                    nf = NCHF[h0]
                    hastail = TAIL[h0]
                    po0 = pspool.tile([P, THALF], F32, name="po", tag="po",
                                      bufs=2)
                    po1 = pspool.tile([P, THALF], F32, name="po", tag="po",
                                      bufs=2)

                    def pv(h, po_, exs_, j, last):
                        for tq in range(2):
                            nc.tensor.matmul(
                                po_[0:65, tq * 512:(tq + 1) * 512],
                                lhsT=va_h(h, j),
                                rhs=exs_[j][:, tq * 512:(tq + 1) * 512],
                                start=(j == 0 and not hastail), stop=last)

                    # packed tail FIRST so it flows through the normal
                    # scores->exp->PV software pipeline: head-even lanes at
                    # partitions 0:64, head-odd at 64:128; the 4 score MMs
                    # land on 4 distinct PE quadrants and run concurrently,
                    # one exp covers both heads, and the tail PVs open the
                    # po accumulations (start=True).
                    ext = None
                    if hastail:
                        ktA, ktB, kc0 = kt_pair(pr)
                        ps = pspool.tile([P, THALF], F32, name="ps",
                                         tag="ps", bufs=2)
                        for hh, rbase in ((h0, 0), (h1, 64)):
                            for tq in range(2):
                                if tq == 0:
                                    kt, kr = ktA, (hh % 2) * 64
                                else:
                                    kt, kr = ktB, (1 - hh % 2) * 64
                                qt, qr = qT_h(hh, tq)
                                nc.tensor.matmul(
                                    ps[rbase:rbase + 64,
                                       tq * 512:(tq + 1) * 512],
                                    lhsT=kt[kr:kr + 64, kc0:kc0 + 64],
                                    rhs=qt[qr:qr + 64, t0 + tq * 512:
                                           t0 + (tq + 1) * 512],
                                    start=True, stop=True)
                        ext = epool.tile([P, THALF], F16, name="ex",
                                         tag="ex", bufs=6)
                        nc.scalar.activation(
                            ext[:], ps[:], mybir.ActivationFunctionType.Exp,
                            bias=0.0, scale=SCALE)

                    def tail_pv(po_, rbase, vc, last):
                        for tq in range(2):
                            nc.tensor.matmul(
                                po_[0:65, tq * 512:(tq + 1) * 512],
                                lhsT=vat_pair(pr)[0][rbase:rbase + 64,
                                                     vc:vc + 65],
                                rhs=ext[rbase:rbase + 64,
                                        tq * 512:(tq + 1) * 512],
                                start=True, stop=last)

                    def full_head(h, po_, first):
                        exs = []
                        for j in range(nf):
                            ps = pspool.tile([P, THALF], F32, name="ps",
                                             tag="ps", bufs=2)
                            for tq in range(2):
                                kt, kr = kT_h(h, tq)
                                qt, qr = qT_h(h, tq)
                                nc.tensor.matmul(
                                    ps[:, tq * 512:(tq + 1) * 512],
                                    lhsT=kt[kr:kr + 64, j * P:(j + 1) * P],
                                    rhs=qt[qr:qr + 64, t0 + tq * 512:
                                           t0 + (tq + 1) * 512],
                                    start=True, stop=True)
                            ex = epool.tile([P, THALF], F16, name="ex",
                                            tag="ex", bufs=6)
                            nc.scalar.activation(
                                ex[:], ps[:],
                                mybir.ActivationFunctionType.Exp,
                                bias=0.0, scale=SCALE)
                            exs.append(ex)
                            if j == 0 and hastail:
                                # lag-1 slot: emit the pair's tail PVs here
                                vc0 = vat_pair(pr)[1]
                                if first:
                                    tail_pv(po0, 0, vc0, last=False)
                                    tail_pv(po1, 64, vc0 + 65, last=False)
                            if j >= 1:
                                pv(h, po_, exs, j - 1, last=False)
                            # inject t-half-0 projection chunks into the
                            # long heads of t-half 1; j=2/6 are far enough
                            # apart for the ACT backlog to recover
                            if th == 1 and h >= 4 and j in (2, 6):
                                proj_chunk(2 * (h - 4) + (0 if j == 2 else 1))
                        if nf >= 1:
                            pv(h, po_, exs, nf - 1, last=True)

                    full_head(h0, po0, first=True)
                    full_head(h1, po1, first=False)
                    if hastail and nf == 0:
                        vc0 = vat_pair(pr)[1]
                        tail_pv(po0, 0, vc0, last=True)
                        tail_pv(po1, 64, vc0 + 65, last=True)

                    # bridge the final normalization chain with dummy
                    # matmuls so the PE stays warm into the projection tail
                    if th == 1 and pr == 3:
                        norm(h0, po0, t0)
                        norm(h1, po1, t0)
                        warm_burst(14)
                    else:
                        norm(h0, po0, t0)
                        norm(h1, po1, t0)
            for tq in range(8, 16):
                proj_chunk(tq)

    nc.compile()
    return nc


_PROGRAMS = {}


def _get_program(key):
    if key not in _PROGRAMS:
        _PROGRAMS[key] = build_program(*key)
    return _PROGRAMS[key]


def _swap_halves(m):
    # [128k, N] -> swap the two 64-row halves within each 128-row block
    blocks = [m[i:i + P] for i in range(0, m.shape[0], P)]
    return np.vstack([np.vstack([b[64:P], b[0:64]]) for b in blocks])


def _prep_core_inputs(query, key, value, wT, pb, keeps, cfg):
    nf4, nf2, nf1, t4, t2, t1 = cfg
    NF = {4: max(nf4, 1), 2: max(nf2, 1), 1: max(nf1, 1)}
    NFR = {4: nf4, 2: nf2, 1: nf1}
    ins = []
    for b in range(B):
        qb = np.ascontiguousarray(query[:, b, :].T).astype(np.float16)
        qbd = np.vstack([qb, _swap_halves(qb)])

        def build_k(sub, idx, c0, c1, ks):
            z = np.zeros((P, NF[ks] * P), dtype=np.float16)
            g = sub[idx[:NFR[ks] * P]]
            n = g.shape[0]
            z[:, 0:n] = g[:, c0:c1].T.astype(np.float16)
            return np.vstack([z, _swap_halves(z)])

        def build_va(sub, idx, heads, W, ks):
            g = sub[idx[:NFR[ks] * P]]
            z = np.zeros((P, NF[ks] * W), dtype=np.float16)
            for j in range(NF[ks]):
                seg = g[j * P:(j + 1) * P]
                m = seg.shape[0]
                if m == 0:
                    break
                for i, h in enumerate(heads):
                    z[0:m, j * W + i * 65: j * W + i * 65 + 64] = \
                        seg[:, h * 64:(h + 1) * 64].astype(np.float16)
                    z[0:m, j * W + i * 65 + 64] = 1.0
            return z

        def build_kt(sub, idx, pairs, ks):
            # [2P, 64*len(pairs)]: per pair, head-even dims on rows 0:64,
            # head-odd on 64:128, tail lanes as columns; plus swapped copy
            z = np.zeros((P, 64 * len(pairs)), dtype=np.float16)
            g = sub[idx[NFR[ks] * P:]]
            n = g.shape[0]
            for i, (ha, hb) in enumerate(pairs):
                if n:
                    z[0:64, i * 64:i * 64 + n] = \
                        g[:, ha * 64:(ha + 1) * 64].T.astype(np.float16)
                    z[64:P, i * 64:i * 64 + n] = \
                        g[:, hb * 64:(hb + 1) * 64].T.astype(np.float16)
            return np.vstack([z, _swap_halves(z)])

        def build_vat(sub, idx, pairs, ks):
            z = np.zeros((P, 130 * len(pairs)), dtype=np.float16)
            g = sub[idx[NFR[ks] * P:]]
            n = g.shape[0]
            for i, (ha, hb) in enumerate(pairs):
                if n:
                    z[0:n, i * 130:i * 130 + 64] = \
                        g[:, ha * 64:(ha + 1) * 64].astype(np.float16)
                    z[0:n, i * 130 + 64] = 1.0
                    z[64:64 + n, i * 130 + 65:i * 130 + 129] = \
                        g[:, hb * 64:(hb + 1) * 64].astype(np.float16)
                    z[64:64 + n, i * 130 + 129] = 1.0
            return z

        kb, vb = key[:, b, :], value[:, b, :]
        i4, i2, i1 = keeps[4][b], keeps[2][b], keeps[1][b]
        ins.append({
            "qT": qbd,
            "k4T": build_k(kb[::4], i4, 0, 128, 4),
            "k2T": build_k(kb[::2], i2, 128, 256, 2),
            "k1Ta": build_k(kb, i1, 256, 384, 1),
            "k1Tb": build_k(kb, i1, 384, 512, 1),
            "va4": build_va(vb[::4], i4, [0, 1], 130, 4),
            "va2": build_va(vb[::2], i2, [2, 3], 130, 2),
            "va1": build_va(vb, i1, [4, 5, 6, 7], 260, 1),
            "kt4": build_kt(kb[::4], i4, [(0, 1)], 4),
            "kt2": build_kt(kb[::2], i2, [(2, 3)], 2),
            "kt1": build_kt(kb, i1, [(4, 5), (6, 7)], 1),
            "vat4": build_vat(vb[::4], i4, [(0, 1)], 4),
            "vat2": build_vat(vb[::2], i2, [(2, 3)], 2),
            "vat1": build_vat(vb, i1, [(4, 5), (6, 7)], 1),
            "wT": wT, "pb": pb,
        })
    return ins


def kernel(query, key, value, attn_mask, proj_w, proj_b, _trace=False,
           **run_kwargs):
    query = np.asarray(query, dtype=np.float32)
    key = np.asarray(key, dtype=np.float32)
    value = np.asarray(value, dtype=np.float32)
    mask = np.asarray(attn_mask).astype(bool)
    wT = np.ascontiguousarray(
        np.asarray(proj_w, dtype=np.float32).T).astype(np.float16)
    pb = np.ascontiguousarray(
        np.asarray(proj_b, dtype=np.float32).reshape(1, E))

    keeps = {ks: [np.flatnonzero(~mask[b, ::ks]) for b in range(B)]
             for ks in (4, 2, 1)}
    cfg = []
    for ks in (4, 2, 1):
        mx = max(len(keeps[ks][b]) for b in range(B))
        nf, lt = divmod(mx, P)
        if lt > 64:
            # tail too wide to pack two heads side by side: pad to a full
            # chunk instead
            nf, lt = nf + 1, 0
        cfg.append((nf, lt > 0))
    cfg = (cfg[0][0], cfg[1][0], cfg[2][0], cfg[0][1], cfg[1][1], cfg[2][1])

    nc = _get_program(cfg)
    ins = _prep_core_inputs(query, key, value, wT, pb, keeps, cfg)
    res = run_bass_kernel_spmd(nc, ins, list(range(B)), trace=_trace,
                               **run_kwargs)
    outs = [np.asarray(res.results[b]["out"]) for b in range(B)]
    full = np.concatenate(outs, axis=0)          # (B*T, E), b-major rows
    result = full.reshape(T, B, E)
    if _trace:
        return result, res
    return result


# revision 17
# speedup vs baseline: 1.1293x; 1.0345x over previous
"""Sparse multi-head attention (per-head strided K/V subsampling) for trn2.

Problem (hardcoded):
  query/key/value: (2048, 8, 512) f32, attn_mask: (8, 2048) bool,
  proj_w: (512, 512), proj_b: (512,).
  Per head h (8 heads, head_dim 64) with stride ksz in [4,4,2,2,1,1,1,1]:
    scores = q_h @ k_h[::ksz].T * 0.125, masked softmax over subsampled keys,
    o_h = softmax @ v_h[::ksz].
  Reference then does a RAW reshape (B,T,D)->(T,B,D) per head before concat +
  out-projection.  That reshape is a pure row permutation of the flattened
  (B*T, 512) matrix, so computing per-(batch,head) attention in (t, d) layout,
  concatenating per batch, projecting, stacking batches, and reshaping
  (B*T, 512) -> (T, B, 512) reproduces it exactly.

Sharding: batch-parallel, one batch element per NeuronCore (8 cores).

Device/layout design (measured-on-HW rationale):
  - mask-gather on the host: masked keys contribute exactly zero, so only
    unmasked subsampled keys are shipped (~50%). Pad rows are all-zero
    INCLUDING the ones-column of the V-augmented matrix, so pads add 0 to
    both numerator and denominator (their exp(0)=1 hits zero V rows).
  - all matmul operands fp16 (f32r streams ~3x slower per row on real HW).
  - scores computed transposed (s on partitions, t free); V augmented with a
    ones column so one accumulating matmul produces both the unnormalized
    output (rows 0:64 of po) and the softmax denominator (row 64).
  - qT/kT are stored TWICE, with the 64-row halves swapped in the copy: the
    two score matmuls of a chunk then run on opposite PE row groups, so they
    execute concurrently (row tiling) and the second LDWEIGHTS overlaps the
    first matmul instead of serializing.
  - the heads of a pair share their keep-set, so their sub-128 tail lanes
    are PACKED into one chunk (head A at partitions 0:64, head B at 64:128,
    exploiting row+col PE tiling) - one ACTIVATE instead of two for the
    tails, per pair per t-half.
  - exp fused on ACT: ex = exp(0.125 * scores) in one [128, 1024] ACTIVATE
    per chunk/t-half (ACT is the pacing engine: (N+352)/1.2ns per inst).
  - normalization: po rows are copied to SBUF in two DVE ops (releases the
    PSUM accumulator ~1.5us after the last PV matmul), then
    reciprocal_approx_fast (custom DVE ops need partition-0 SBUF inputs) +
    gpsimd partition-broadcast + DVE multiply, all off the critical path.
  - out-projection with host-side proj_w.T; bias added via DVE tensor_add
    with a pre-broadcast bias tile during the PSUM->SBUF copy. Projection
    chunks for t-half 0 are injected mid-head into the long heads of
    t-half 1 (j=2 and j=6, far enough apart for the ACT backlog to recover).
  - dummy matmul bursts keep the PE HAM clock gate warm: once during the
    initial DMA wait, once while the final head's normalization chain runs
    (so the projection tail executes at 2.4 GHz).
"""

import numpy as np

import concourse.bass as bass
import concourse.tile as tile
from concourse import bacc, mybir
from concourse.bass_utils import run_bass_kernel_spmd

T = 2048
B = 8
E = 512
H = 8
D = 64
KS = [4, 4, 2, 2, 1, 1, 1, 1]
SCALE = 0.125
P = 128
THALF = 1024
F32 = mybir.dt.float32
F16 = mybir.dt.float16


def build_program(nf4, nf2, nf1, t4, t2, t1):
    # nfX: number of full 128-lane chunks per stride group; tX: group has a
    # packed <=64-lane tail chunk shared by the head pair
    N4, N2, N1 = max(nf4, 1) * P, max(nf2, 1) * P, max(nf1, 1) * P
    nc = bacc.Bacc("TRN2", target_bir_lowering=False, debug=False, num_devices=B)

    qT = nc.dram_tensor("qT", [2 * E, T], F16, kind="ExternalInput")
    k4T = nc.dram_tensor("k4T", [2 * P, N4], F16, kind="ExternalInput")
    k2T = nc.dram_tensor("k2T", [2 * P, N2], F16, kind="ExternalInput")
    k1Ta = nc.dram_tensor("k1Ta", [2 * P, N1], F16, kind="ExternalInput")
    k1Tb = nc.dram_tensor("k1Tb", [2 * P, N1], F16, kind="ExternalInput")
    va4 = nc.dram_tensor("va4", [P, max(nf4, 1) * 130], F16,
                         kind="ExternalInput")
    va2 = nc.dram_tensor("va2", [P, max(nf2, 1) * 130], F16,
                         kind="ExternalInput")
    va1 = nc.dram_tensor("va1", [P, max(nf1, 1) * 260], F16,
                         kind="ExternalInput")
    kt4 = nc.dram_tensor("kt4", [2 * P, 64], F16, kind="ExternalInput")
    kt2 = nc.dram_tensor("kt2", [2 * P, 64], F16, kind="ExternalInput")
    kt1 = nc.dram_tensor("kt1", [2 * P, 128], F16, kind="ExternalInput")
    vat4 = nc.dram_tensor("vat4", [P, 130], F16, kind="ExternalInput")
    vat2 = nc.dram_tensor("vat2", [P, 130], F16, kind="ExternalInput")
    vat1 = nc.dram_tensor("vat1", [P, 260], F16, kind="ExternalInput")
    wT = nc.dram_tensor("wT", [E, E], F16, kind="ExternalInput")
    pb = nc.dram_tensor("pb", [1, E], F32, kind="ExternalInput")
    out = nc.dram_tensor("out", [T, E], F32, kind="ExternalOutput")

    NCHF = [nf4, nf4, nf2, nf2, nf1, nf1, nf1, nf1]
    TAIL = [t4, t4, t2, t2, t1, t1, t1, t1]

    with tile.TileContext(nc) as tc:
        with (
            tc.tile_pool(name="const", bufs=1) as cpool,
            tc.tile_pool(name="exp", bufs=4) as epool,
            tc.tile_pool(name="norm", bufs=2) as npool,
            tc.tile_pool(name="outsb", bufs=4) as opool,
            tc.tile_pool(name="psA", bufs=1, space="PSUM") as pspool,
        ):
            # ---- persistent SBUF loads (ordered by first use) ----
            qTA_sb, qTB_sb = [], []
            for p_ in range(4):
                qTA_sb.append(cpool.tile([P, T], F16, name=f"qTA{p_}",
                                         tag=f"qTA{p_}"))
                qTB_sb.append(cpool.tile([P, T], F16, name=f"qTB{p_}",
                                         tag=f"qTB{p_}"))
            nc.sync.dma_start(qTA_sb[0][:], qT.ap()[0:P, :])
            nc.sync.dma_start(qTB_sb[0][:], qT.ap()[E:E + P, :])

            def kpair(name, dram, N):
                a = cpool.tile([P, N], F16, name=name + "a", tag=name + "a")
                b = cpool.tile([P, N], F16, name=name + "b", tag=name + "b")
                nc.sync.dma_start(a[:], dram.ap()[0:P, :])
                nc.sync.dma_start(b[:], dram.ap()[P:2 * P, :])
                return a, b

            kt4A, kt4B = kpair("kt4", kt4, 64)
            vat4_sb = cpool.tile([P, 130], F16, name="vat4s", tag="vat4s")
            nc.sync.dma_start(vat4_sb[:], vat4.ap())
            k4A, k4B = kpair("k4", k4T, N4)
            va4_sb = cpool.tile([P, max(nf4, 1) * 130], F16, name="va4s",
                                tag="va4s")
            nc.sync.dma_start(va4_sb[:], va4.ap())
            k2A, k2B = kpair("k2", k2T, N2)
            nc.sync.dma_start(qTA_sb[1][:], qT.ap()[P:2 * P, :])
            nc.sync.dma_start(qTB_sb[1][:], qT.ap()[E + P:E + 2 * P, :])
            va2_sb = cpool.tile([P, max(nf2, 1) * 130], F16, name="va2s",
                                tag="va2s")
            nc.sync.dma_start(va2_sb[:], va2.ap())
            kt2A, kt2B = kpair("kt2", kt2, 64)
            vat2_sb = cpool.tile([P, 130], F16, name="vat2s", tag="vat2s")
            nc.sync.dma_start(vat2_sb[:], vat2.ap())
            k1aA, k1aB = kpair("k1a", k1Ta, N1)
            nc.sync.dma_start(qTA_sb[2][:], qT.ap()[2 * P:3 * P, :])
            nc.sync.dma_start(qTB_sb[2][:], qT.ap()[E + 2 * P:E + 3 * P, :])
            k1bA, k1bB = kpair("k1b", k1Tb, N1)
            nc.sync.dma_start(qTA_sb[3][:], qT.ap()[3 * P:4 * P, :])
            nc.sync.dma_start(qTB_sb[3][:], qT.ap()[E + 3 * P:E + 4 * P, :])
            va1_sb = cpool.tile([P, max(nf1, 1) * 260], F16, name="va1s",
                                tag="va1s")
            nc.sync.dma_start(va1_sb[:], va1.ap())
            kt1A, kt1B = kpair("kt1", kt1, 128)
            vat1_sb = cpool.tile([P, 260], F16, name="vat1s", tag="vat1s")
            nc.sync.dma_start(vat1_sb[:], vat1.ap())
            wT_sb = []
            for i in range(4):
                t_ = cpool.tile([P, E], F16, name=f"wT{i}", tag=f"wT{i}")
                nc.sync.dma_start(t_[:], wT.ap()[i * P:(i + 1) * P, :])
                wT_sb.append(t_)
            pb_sb = cpool.tile([1, E], F32, name="pbs", tag="pbs")
            nc.sync.dma_start(pb_sb[:], pb.ap())
            pbb_sb = cpool.tile([P, E], F32, name="pbb", tag="pbb")
            nc.gpsimd.partition_broadcast(pbb_sb[:], pb_sb[:])

            # ---- PE warm-up burst ----
            wu_sb = cpool.tile([64, 512], F16, name="wu", tag="wu")
            nc.gpsimd.memset(wu_sb[:], 0.0)

            def warm_burst(n):
                wps = pspool.tile([P, THALF], F32, name="ps", tag="ps",
                                  bufs=2)
                for _ in range(n):
                    nc.tensor.matmul(
                        wps[0:4, 0:512], lhsT=wu_sb[:, 0:4], rhs=wu_sb[:],
                        start=True, stop=True)

            warm_burst(8)

            # per-head views: (tile, row0) for tq0 (natural) / tq1 (swapped)
            def kT_h(h, tq):
                A, Bt = [(k4A, k4B), (k4A, k4B), (k2A, k2B), (k2A, k2B),
                         (k1aA, k1aB), (k1aA, k1aB), (k1bA, k1bB),
                         (k1bA, k1bB)][h]
                if tq == 0:
                    return A, (h % 2) * 64
                return Bt, (1 - h % 2) * 64

            def qT_h(h, tq):
                if tq == 0:
                    return qTA_sb[h // 2], (h % 2) * 64
                return qTB_sb[h // 2], (1 - h % 2) * 64

            def va_h(h, j):
                if h < 2:
                    return va4_sb[:, j * 130 + h * 65: j * 130 + h * 65 + 65]
                if h < 4:
                    return va2_sb[:, j * 130 + (h - 2) * 65:
                                  j * 130 + (h - 2) * 65 + 65]
                return va1_sb[:, j * 260 + (h - 4) * 65:
                              j * 260 + (h - 4) * 65 + 65]

            def kt_pair(pr):
                # (Atile, Btile, col0) for the packed tail of head pair pr
                if pr == 0:
                    return kt4A, kt4B, 0
                if pr == 1:
                    return kt2A, kt2B, 0
                if pr == 2:
                    return kt1A, kt1B, 0
                return kt1A, kt1B, 64

            def vat_pair(pr):
                # (tile, col0): head even at rows 0:64 cols c0:c0+65,
                # head odd at rows 64:128 cols c0+65:c0+130
                if pr == 0:
                    return vat4_sb, 0
                if pr == 1:
                    return vat2_sb, 0
                if pr == 2:
                    return vat1_sb, 0
                return vat1_sb, 130

            # transposed normalized head outputs (fp16), feeding proj
            oT_sb = []
            for p_ in range(4):
                t_ = cpool.tile([P, T], F16, name=f"oT{p_}", tag=f"oT{p_}")
                oT_sb.append(t_)

            def proj_chunk(tq):
                pp_full = pspool.tile([P, THALF], F32, name="pp", tag="ps",
                                      bufs=2)
                pp = pp_full[:, 0:E]
                for i in range(4):
                    nc.tensor.matmul(
                        pp, lhsT=oT_sb[i][:, tq * P:(tq + 1) * P],
                        rhs=wT_sb[i][:], start=(i == 0), stop=(i == 3))
                ot = opool.tile([P, E], F32, name="ot", tag="ot")
                nc.vector.tensor_add(ot[:], pp, pbb_sb[:])
                nc.sync.dma_start(out.ap()[tq * P:(tq + 1) * P, :], ot[:])

            def norm(h, po_, t0):
                # two DVE copies release po quickly; recip/broadcast/multiply
                # run from SBUF off the fast path. den must be a partition-0
                # tile: custom DVE ops misread non-zero base partitions.
                oU = npool.tile([64, THALF], F32, name="oU", tag="oU")
                nc.vector.tensor_copy(oU[:], po_[0:64, :])
                den = npool.tile([1, THALF], F32, name="den", tag="den")
                nc.vector.tensor_copy(den[:], po_[64:65, :])
                rec = npool.tile([1, THALF], F32, name="rec", tag="rec")
                nc.vector.reciprocal_approx_fast(rec[:], den[:])
                rbc = npool.tile([64, THALF], F32, name="rbc", tag="rbc")
                nc.gpsimd.partition_broadcast(rbc[:], rec[:])
                r0 = (h % 2) * 64
                nc.vector.tensor_mul(
                    oT_sb[h // 2][r0:r0 + 64, t0:t0 + THALF], oU[:], rbc[:])

            # ---- attention main loop ----
            for th in range(2):
                t0 = th * THALF
                for pr in range(4):
                    h0, h1 = 2 * pr, 2 * pr + 1
                    nf = NCHF[h0]
                    hastail = TAIL[h0]
                    po0 = pspool.tile([P, THALF], F32, name="po", tag="po",
                                      bufs=2)
                    po1 = pspool.tile([P, THALF], F32, name="po", tag="po",
                                      bufs=2)

                    def pv(h, po_, exs_, j, last):
                        for tq in range(2):
                            nc.tensor.matmul(
                                po_[0:65, tq * 512:(tq + 1) * 512],
                                lhsT=va_h(h, j),
                                rhs=exs_[j][:, tq * 512:(tq + 1) * 512],
                                start=(j == 0 and not hastail), stop=last)

                    # packed tail FIRST so it flows through the normal
                    # scores->exp->PV software pipeline: head-even lanes at
                    # partitions 0:64, head-odd at 64:128; the 4 score MMs
                    # land on 4 distinct PE quadrants and run concurrently,
                    # one exp covers both heads, and the tail PVs open the
                    # po accumulations (start=True).
                    ext = None
                    if hastail:
                        ktA, ktB, kc0 = kt_pair(pr)
                        ps = pspool.tile([P, THALF], F32, name="ps",
                                         tag="ps", bufs=2)
                        for hh, rbase in ((h0, 0), (h1, 64)):
                            for tq in range(2):
                                if tq == 0:
                                    kt, kr = ktA, (hh % 2) * 64
                                else:
                                    kt, kr = ktB, (1 - hh % 2) * 64
                                qt, qr = qT_h(hh, tq)
                                nc.tensor.matmul(
                                    ps[rbase:rbase + 64,
                                       tq * 512:(tq + 1) * 512],
                                    lhsT=kt[kr:kr + 64, kc0:kc0 + 64],
                                    rhs=qt[qr:qr + 64, t0 + tq * 512:
                                           t0 + (tq + 1) * 512],
                                    start=True, stop=True)
                        ext = epool.tile([P, THALF], F16, name="ex",
                                         tag="ex", bufs=6)
                        nc.scalar.activation(
                            ext[:], ps[:], mybir.ActivationFunctionType.Exp,
                            bias=0.0, scale=SCALE)

                    def tail_pv(po_, rbase, vc, last):
                        for tq in range(2):
                            nc.tensor.matmul(
                                po_[0:65, tq * 512:(tq + 1) * 512],
                                lhsT=vat_pair(pr)[0][rbase:rbase + 64,
                                                     vc:vc + 65],
                                rhs=ext[rbase:rbase + 64,
                                        tq * 512:(tq + 1) * 512],
                                start=True, stop=last)

                    def full_head(h, po_, first):
                        exs = []
                        for j in range(nf):
                            ps = pspool.tile([P, THALF], F32, name="ps",
                                             tag="ps", bufs=2)
                            for tq in range(2):
                                kt, kr = kT_h(h, tq)
                                qt, qr = qT_h(h, tq)
                                nc.tensor.matmul(
                                    ps[:, tq * 512:(tq + 1) * 512],
                                    lhsT=kt[kr:kr + 64, j * P:(j + 1) * P],
                                    rhs=qt[qr:qr + 64, t0 + tq * 512:
                                           t0 + (tq + 1) * 512],
                                    start=True, stop=True)
                            ex = epool.tile([P, THALF], F16, name="ex",
                                            tag="ex", bufs=6)
                            nc.scalar.activation(
                                ex[:], ps[:],
                                mybir.ActivationFunctionType.Exp,
                                bias=0.0, scale=SCALE)
                            exs.append(ex)
                            if j == 0 and hastail:
                                # lag-1 slot: emit the pair's tail PVs here
                                vc0 = vat_pair(pr)[1]
                                if first:
                                    tail_pv(po0, 0, vc0, last=False)
                                    tail_pv(po1, 64, vc0 + 65, last=False)
                            if j >= 1:
                                pv(h, po_, exs, j - 1, last=False)
                            # inject t-half-0 projection chunks into the
                            # long heads of t-half 1; j=2/6 are far enough
                            # apart for the ACT backlog to recover
                            if th == 1 and h >= 4 and j in (2, 6):
                                proj_chunk(2 * (h - 4) + (0 if j == 2 else 1))
                        if nf >= 1:
                            pv(h, po_, exs, nf - 1, last=True)

                    full_head(h0, po0, first=True)
                    full_head(h1, po1, first=False)
                    if hastail and nf == 0:
                        vc0 = vat_pair(pr)[1]
                        tail_pv(po0, 0, vc0, last=True)
                        tail_pv(po1, 64, vc0 + 65, last=True)

                    # bridge the final normalization chain with dummy
                    # matmuls so the PE stays warm into the projection tail
                    if th == 1 and pr == 3:
                        norm(h0, po0, t0)
                        norm(h1, po1, t0)
                        warm_burst(14)
                    else:
                        norm(h0, po0, t0)
                        norm(h1, po1, t0)
            for tq in range(8, 16):
                proj_chunk(tq)

    nc.compile()
    return nc


_PROGRAMS = {}


def _get_program(key):
    if key not in _PROGRAMS:
        _PROGRAMS[key] = build_program(*key)
    return _PROGRAMS[key]


def _swap_halves(m):
    # [128k, N] -> swap the two 64-row halves within each 128-row block
    blocks = [m[i:i + P] for i in range(0, m.shape[0], P)]
    return np.vstack([np.vstack([b[64:P], b[0:64]]) for b in blocks])


def _prep_core_inputs(query, key, value, wT, pb, keeps, cfg):
    nf4, nf2, nf1, t4, t2, t1 = cfg
    NF = {4: max(nf4, 1), 2: max(nf2, 1), 1: max(nf1, 1)}
    NFR = {4: nf4, 2: nf2, 1: nf1}
    ins = []
    for b in range(B):
        qb = np.ascontiguousarray(query[:, b, :].T).astype(np.float16)
        qbd = np.vstack([qb, _swap_halves(qb)])

        def build_k(sub, idx, c0, c1, ks):
            z = np.zeros((P, NF[ks] * P), dtype=np.float16)
            g = sub[idx[:NFR[ks] * P]]
            n = g.shape[0]
            z[:, 0:n] = g[:, c0:c1].T.astype(np.float16)
            return np.vstack([z, _swap_halves(z)])

        def build_va(sub, idx, heads, W, ks):
            g = sub[idx[:NFR[ks] * P]]
            z = np.zeros((P, NF[ks] * W), dtype=np.float16)
            for j in range(NF[ks]):
                seg = g[j * P:(j + 1) * P]
                m = seg.shape[0]
                if m == 0:
                    break
                for i, h in enumerate(heads):
                    z[0:m, j * W + i * 65: j * W + i * 65 + 64] = \
                        seg[:, h * 64:(h + 1) * 64].astype(np.float16)
                    z[0:m, j * W + i * 65 + 64] = 1.0
            return z

        def build_kt(sub, idx, pairs, ks):
            # [2P, 64*len(pairs)]: per pair, head-even dims on rows 0:64,
            # head-odd on 64:128, tail lanes as columns; plus swapped copy
            z = np.zeros((P, 64 * len(pairs)), dtype=np.float16)
            g = sub[idx[NFR[ks] * P:]]
            n = g.shape[0]
            for i, (ha, hb) in enumerate(pairs):
                if n:
                    z[0:64, i * 64:i * 64 + n] = \
                        g[:, ha * 64:(ha + 1) * 64].T.astype(np.float16)
                    z[64:P, i * 64:i * 64 + n] = \
                        g[:, hb * 64:(hb + 1) * 64].T.astype(np.float16)
            return np.vstack([z, _swap_halves(z)])

        def build_vat(sub, idx, pairs, ks):
            z = np.zeros((P, 130 * len(pairs)), dtype=np.float16)
            g = sub[idx[NFR[ks] * P:]]
            n = g.shape[0]
            for i, (ha, hb) in enumerate(pairs):
                if n:
                    z[0:n, i * 130:i * 130 + 64] = \
                        g[:, ha * 64:(ha + 1) * 64].astype(np.float16)
                    z[0:n, i * 130 + 64] = 1.0
                    z[64:64 + n, i * 130 + 65:i * 130 + 129] = \
                        g[:, hb * 64:(hb + 1) * 64].astype(np.float16)
                    z[64:64 + n, i * 130 + 129] = 1.0
            return z

        kb, vb = key[:, b, :], value[:, b, :]
        i4, i2, i1 = keeps[4][b], keeps[2][b], keeps[1][b]
        ins.append({
            "qT": qbd,
            "k4T": build_k(kb[::4], i4, 0, 128, 4),
            "k2T": build_k(kb[::2], i2, 128, 256, 2),
            "k1Ta": build_k(kb, i1, 256, 384, 1),
            "k1Tb": build_k(kb, i1, 384, 512, 1),
            "va4": build_va(vb[::4], i4, [0, 1], 130, 4),
            "va2": build_va(vb[::2], i2, [2, 3], 130, 2),
            "va1": build_va(vb, i1, [4, 5, 6, 7], 260, 1),
            "kt4": build_kt(kb[::4], i4, [(0, 1)], 4),
            "kt2": build_kt(kb[::2], i2, [(2, 3)], 2),
            "kt1": build_kt(kb, i1, [(4, 5), (6, 7)], 1),
            "vat4": build_vat(vb[::4], i4, [(0, 1)], 4),
            "vat2": build_vat(vb[::2], i2, [(2, 3)], 2),
            "vat1": build_vat(vb, i1, [(4, 5), (6, 7)], 1),
            "wT": wT, "pb": pb,
        })
    return ins


def kernel(query, key, value, attn_mask, proj_w, proj_b, _trace=False,
           **run_kwargs):
    query = np.asarray(query, dtype=np.float32)
    key = np.asarray(key, dtype=np.float32)
    value = np.asarray(value, dtype=np.float32)
    mask = np.asarray(attn_mask).astype(bool)
    wT = np.ascontiguousarray(
        np.asarray(proj_w, dtype=np.float32).T).astype(np.float16)
    pb = np.ascontiguousarray(
        np.asarray(proj_b, dtype=np.float32).reshape(1, E))

    keeps = {ks: [np.flatnonzero(~mask[b, ::ks]) for b in range(B)]
             for ks in (4, 2, 1)}
    import os
    notails = os.environ.get("KERNEL_NOTAILS") == "1"
    cfg = []
    for ks in (4, 2, 1):
        mx = max(len(keeps[ks][b]) for b in range(B))
        nf, lt = divmod(mx, P)
        if notails and lt:
            nf, lt = nf + 1, 0
        if lt > 64:
            # tail too wide to pack two heads side by side: pad to a full
            # chunk instead
            nf, lt = nf + 1, 0
        cfg.append((nf, lt > 0))
    cfg = (cfg[0][0], cfg[1][0], cfg[2][0], cfg[0][1], cfg[1][1], cfg[2][1])

    nc = _get_program(cfg)
    ins = _prep_core_inputs(query, key, value, wT, pb, keeps, cfg)
    res = run_bass_kernel_spmd(nc, ins, list(range(B)), trace=_trace,
                               **run_kwargs)
    outs = [np.asarray(res.results[b]["out"]) for b in range(B)]
    full = np.concatenate(outs, axis=0)          # (B*T, E), b-major rows
    result = full.reshape(T, B, E)
    if _trace:
        return result, res
    return result


# revision 18
# speedup vs baseline: 1.1402x; 1.0096x over previous
"""Sparse multi-head attention (per-head strided K/V subsampling) for trn2.

Problem (hardcoded):
  query/key/value: (2048, 8, 512) f32, attn_mask: (8, 2048) bool,
  proj_w: (512, 512), proj_b: (512,).
  Per head h (8 heads, head_dim 64) with stride ksz in [4,4,2,2,1,1,1,1]:
    scores = q_h @ k_h[::ksz].T * 0.125, masked softmax over subsampled keys,
    o_h = softmax @ v_h[::ksz].
  Reference then does a RAW reshape (B,T,D)->(T,B,D) per head before concat +
  out-projection.  That reshape is a pure row permutation of the flattened
  (B*T, 512) matrix, so computing per-(batch,head) attention in (t, d) layout,
  concatenating per batch, projecting, stacking batches, and reshaping
  (B*T, 512) -> (T, B, 512) reproduces it exactly.

Sharding: batch-parallel, one batch element per NeuronCore (8 cores).

Device/layout design (measured-on-HW rationale):
  - mask-gather on the host: masked keys contribute exactly zero, so only
    unmasked subsampled keys are shipped (~50%). Pad rows are all-zero
    INCLUDING the ones-column of the V-augmented matrix, so pads add 0 to
    both numerator and denominator (their exp(0)=1 hits zero V rows).
  - all matmul operands fp16 (f32r streams ~3x slower per row on real HW).
  - scores computed transposed (s on partitions, t free); V augmented with a
    ones column so one accumulating matmul produces both the unnormalized
    output (rows 0:64 of po) and the softmax denominator (row 64).
  - qT/kT are stored TWICE, with the 64-row halves swapped in the copy: the
    two score matmuls of a chunk then run on opposite PE row groups, so they
    execute concurrently (row tiling) and the second LDWEIGHTS overlaps the
    first matmul instead of serializing.
  - the heads of a pair share their keep-set, so their sub-128 tail lanes
    are PACKED into one chunk (head A at partitions 0:64, head B at 64:128,
    exploiting row+col PE tiling) - one ACTIVATE instead of two for the
    tails, per pair per t-half.
  - exp fused on ACT: ex = exp(0.125 * scores) in one [128, 1024] ACTIVATE
    per chunk/t-half (ACT is the pacing engine: (N+352)/1.2ns per inst).
  - normalization: po rows are copied to SBUF in two DVE ops (releases the
    PSUM accumulator ~1.5us after the last PV matmul), then
    reciprocal_approx_fast (custom DVE ops need partition-0 SBUF inputs) +
    gpsimd partition-broadcast + DVE multiply, all off the critical path.
  - out-projection with host-side proj_w.T; bias added via DVE tensor_add
    with a pre-broadcast bias tile during the PSUM->SBUF copy. Projection
    chunks for t-half 0 are injected mid-head into the long heads of
    t-half 1 (j=2 and j=6, far enough apart for the ACT backlog to recover).
  - dummy matmul bursts keep the PE HAM clock gate warm: once during the
    initial DMA wait, once while the final head's normalization chain runs
    (so the projection tail executes at 2.4 GHz).
"""

import numpy as np

import concourse.bass as bass
import concourse.tile as tile
from concourse import bacc, mybir
from concourse.bass_utils import run_bass_kernel_spmd

T = 2048
B = 8
E = 512
H = 8
D = 64
KS = [4, 4, 2, 2, 1, 1, 1, 1]
SCALE = 0.125
P = 128
THALF = 1024
F32 = mybir.dt.float32
F16 = mybir.dt.float16


def build_program(nf4, nf2, nf1, t4, t2, t1):
    # nfX: number of full 128-lane chunks per stride group; tX: group has a
    # packed <=64-lane tail chunk shared by the head pair
    N4, N2, N1 = max(nf4, 1) * P, max(nf2, 1) * P, max(nf1, 1) * P
    nc = bacc.Bacc("TRN2", target_bir_lowering=False, debug=False, num_devices=B)

    qT = nc.dram_tensor("qT", [2 * E, T], F16, kind="ExternalInput")
    k4T = nc.dram_tensor("k4T", [2 * P, N4], F16, kind="ExternalInput")
    k2T = nc.dram_tensor("k2T", [2 * P, N2], F16, kind="ExternalInput")
    k1Ta = nc.dram_tensor("k1Ta", [2 * P, N1], F16, kind="ExternalInput")
    k1Tb = nc.dram_tensor("k1Tb", [2 * P, N1], F16, kind="ExternalInput")
    va4 = nc.dram_tensor("va4", [P, max(nf4, 1) * 130], F16,
                         kind="ExternalInput")
    va2 = nc.dram_tensor("va2", [P, max(nf2, 1) * 130], F16,
                         kind="ExternalInput")
    va1 = nc.dram_tensor("va1", [P, max(nf1, 1) * 260], F16,
                         kind="ExternalInput")
    kt4 = nc.dram_tensor("kt4", [2 * P, 64], F16, kind="ExternalInput")
    kt2 = nc.dram_tensor("kt2", [2 * P, 64], F16, kind="ExternalInput")
    kt1 = nc.dram_tensor("kt1", [2 * P, 128], F16, kind="ExternalInput")
    vat4 = nc.dram_tensor("vat4", [P, 130], F16, kind="ExternalInput")
    vat2 = nc.dram_tensor("vat2", [P, 130], F16, kind="ExternalInput")
    vat1 = nc.dram_tensor("vat1", [P, 260], F16, kind="ExternalInput")
    wT = nc.dram_tensor("wT", [E, E], F16, kind="ExternalInput")
    pb = nc.dram_tensor("pb", [1, E], F32, kind="ExternalInput")
    out = nc.dram_tensor("out", [T, E], F32, kind="ExternalOutput")

    NCHF = [nf4, nf4, nf2, nf2, nf1, nf1, nf1, nf1]
    TAIL = [t4, t4, t2, t2, t1, t1, t1, t1]

    with tile.TileContext(nc) as tc:
        with (
            tc.tile_pool(name="const", bufs=1) as cpool,
            tc.tile_pool(name="exp", bufs=4) as epool,
            tc.tile_pool(name="norm", bufs=2) as npool,
            tc.tile_pool(name="outsb", bufs=4) as opool,
            tc.tile_pool(name="psA", bufs=1, space="PSUM") as pspool,
        ):
            # ---- persistent SBUF loads (ordered by first use) ----
            qTA_sb, qTB_sb = [], []
            for p_ in range(4):
                qTA_sb.append(cpool.tile([P, T], F16, name=f"qTA{p_}",
                                         tag=f"qTA{p_}"))
                qTB_sb.append(cpool.tile([P, T], F16, name=f"qTB{p_}",
                                         tag=f"qTB{p_}"))
            nc.sync.dma_start(qTA_sb[0][:], qT.ap()[0:P, :])
            nc.sync.dma_start(qTB_sb[0][:], qT.ap()[E:E + P, :])

            def kpair(name, dram, N):
                a = cpool.tile([P, N], F16, name=name + "a", tag=name + "a")
                b = cpool.tile([P, N], F16, name=name + "b", tag=name + "b")
                nc.sync.dma_start(a[:], dram.ap()[0:P, :])
                nc.sync.dma_start(b[:], dram.ap()[P:2 * P, :])
                return a, b

            kt4A, kt4B = kpair("kt4", kt4, 64)
            vat4_sb = cpool.tile([P, 130], F16, name="vat4s", tag="vat4s")
            nc.sync.dma_start(vat4_sb[:], vat4.ap())
            k4A, k4B = kpair("k4", k4T, N4)
            va4_sb = cpool.tile([P, max(nf4, 1) * 130], F16, name="va4s",
                                tag="va4s")
            nc.sync.dma_start(va4_sb[:], va4.ap())
            k2A, k2B = kpair("k2", k2T, N2)
            nc.sync.dma_start(qTA_sb[1][:], qT.ap()[P:2 * P, :])
            nc.sync.dma_start(qTB_sb[1][:], qT.ap()[E + P:E + 2 * P, :])
            va2_sb = cpool.tile([P, max(nf2, 1) * 130], F16, name="va2s",
                                tag="va2s")
            nc.sync.dma_start(va2_sb[:], va2.ap())
            kt2A, kt2B = kpair("kt2", kt2, 64)
            vat2_sb = cpool.tile([P, 130], F16, name="vat2s", tag="vat2s")
            nc.sync.dma_start(vat2_sb[:], vat2.ap())
            k1aA, k1aB = kpair("k1a", k1Ta, N1)
            nc.sync.dma_start(qTA_sb[2][:], qT.ap()[2 * P:3 * P, :])
            nc.sync.dma_start(qTB_sb[2][:], qT.ap()[E + 2 * P:E + 3 * P, :])
            k1bA, k1bB = kpair("k1b", k1Tb, N1)
            nc.sync.dma_start(qTA_sb[3][:], qT.ap()[3 * P:4 * P, :])
            nc.sync.dma_start(qTB_sb[3][:], qT.ap()[E + 3 * P:E + 4 * P, :])
            va1_sb = cpool.tile([P, max(nf1, 1) * 260], F16, name="va1s",
                                tag="va1s")
            nc.sync.dma_start(va1_sb[:], va1.ap())
            kt1A, kt1B = kpair("kt1", kt1, 128)
            vat1_sb = cpool.tile([P, 260], F16, name="vat1s", tag="vat1s")
            nc.sync.dma_start(vat1_sb[:], vat1.ap())
            wT_sb = []
            for i in range(4):
                t_ = cpool.tile([P, E], F16, name=f"wT{i}", tag=f"wT{i}")
                nc.sync.dma_start(t_[:], wT.ap()[i * P:(i + 1) * P, :])
                wT_sb.append(t_)
            pb_sb = cpool.tile([1, E], F32, name="pbs", tag="pbs")
            nc.sync.dma_start(pb_sb[:], pb.ap())
            pbb_sb = cpool.tile([P, E], F32, name="pbb", tag="pbb")
            nc.gpsimd.partition_broadcast(pbb_sb[:], pb_sb[:])

            # ---- PE warm-up burst ----
            wu_sb = cpool.tile([64, 512], F16, name="wu", tag="wu")
            nc.gpsimd.memset(wu_sb[:], 0.0)

            def warm_burst(n):
                wps = pspool.tile([P, THALF], F32, name="ps", tag="ps",
                                  bufs=2)
                for _ in range(n):
                    nc.tensor.matmul(
                        wps[0:4, 0:512], lhsT=wu_sb[:, 0:4], rhs=wu_sb[:],
                        start=True, stop=True)

            warm_burst(8)

            # per-head views: (tile, row0) for tq0 (natural) / tq1 (swapped)
            def kT_h(h, tq):
                A, Bt = [(k4A, k4B), (k4A, k4B), (k2A, k2B), (k2A, k2B),
                         (k1aA, k1aB), (k1aA, k1aB), (k1bA, k1bB),
                         (k1bA, k1bB)][h]
                if tq == 0:
                    return A, (h % 2) * 64
                return Bt, (1 - h % 2) * 64

            def qT_h(h, tq):
                if tq == 0:
                    return qTA_sb[h // 2], (h % 2) * 64
                return qTB_sb[h // 2], (1 - h % 2) * 64

            def va_h(h, j):
                if h < 2:
                    return va4_sb[:, j * 130 + h * 65: j * 130 + h * 65 + 65]
                if h < 4:
                    return va2_sb[:, j * 130 + (h - 2) * 65:
                                  j * 130 + (h - 2) * 65 + 65]
                return va1_sb[:, j * 260 + (h - 4) * 65:
                              j * 260 + (h - 4) * 65 + 65]

            def kt_pair(pr):
                # (Atile, Btile, col0) for the packed tail of head pair pr
                if pr == 0:
                    return kt4A, kt4B, 0
                if pr == 1:
                    return kt2A, kt2B, 0
                if pr == 2:
                    return kt1A, kt1B, 0
                return kt1A, kt1B, 64

            def vat_pair(pr):
                # (tile, col0): head even at rows 0:64 cols c0:c0+65,
                # head odd at rows 64:128 cols c0+65:c0+130
                if pr == 0:
                    return vat4_sb, 0
                if pr == 1:
                    return vat2_sb, 0
                if pr == 2:
                    return vat1_sb, 0
                return vat1_sb, 130

            # transposed normalized head outputs (fp16), feeding proj
            oT_sb = []
            for p_ in range(4):
                t_ = cpool.tile([P, T], F16, name=f"oT{p_}", tag=f"oT{p_}")
                oT_sb.append(t_)

            def proj_chunk(tq):
                pp_full = pspool.tile([P, THALF], F32, name="pp", tag="ps",
                                      bufs=2)
                pp = pp_full[:, 0:E]
                for i in range(4):
                    nc.tensor.matmul(
                        pp, lhsT=oT_sb[i][:, tq * P:(tq + 1) * P],
                        rhs=wT_sb[i][:], start=(i == 0), stop=(i == 3))
                ot = opool.tile([P, E], F32, name="ot", tag="ot")
                nc.vector.tensor_add(ot[:], pp, pbb_sb[:])
                nc.sync.dma_start(out.ap()[tq * P:(tq + 1) * P, :], ot[:])

            def norm(h, po_, t0):
                # two DVE copies release po quickly; recip/broadcast/multiply
                # run from SBUF off the fast path. den must be a partition-0
                # tile: custom DVE ops misread non-zero base partitions.
                oU = npool.tile([64, THALF], F32, name="oU", tag="oU")
                nc.vector.tensor_copy(oU[:], po_[0:64, :])
                den = npool.tile([1, THALF], F32, name="den", tag="den")
                nc.vector.tensor_copy(den[:], po_[64:65, :])
                rec = npool.tile([1, THALF], F32, name="rec", tag="rec")
                nc.vector.reciprocal_approx_fast(rec[:], den[:])
                rbc = npool.tile([64, THALF], F32, name="rbc", tag="rbc")
                nc.gpsimd.partition_broadcast(rbc[:], rec[:])
                r0 = (h % 2) * 64
                nc.vector.tensor_mul(
                    oT_sb[h // 2][r0:r0 + 64, t0:t0 + THALF], oU[:], rbc[:])

            # ---- attention main loop ----
            for th in range(2):
                t0 = th * THALF
                for pr in range(4):
                    h0, h1 = 2 * pr, 2 * pr + 1
                    nf = NCHF[h0]
                    hastail = TAIL[h0]
                    po0 = pspool.tile([P, THALF], F32, name="po", tag="po",
                                      bufs=2)
                    po1 = pspool.tile([P, THALF], F32, name="po", tag="po",
                                      bufs=2)

                    def pv(h, po_, exs_, j, last):
                        for tq in range(2):
                            nc.tensor.matmul(
                                po_[0:65, tq * 512:(tq + 1) * 512],
                                lhsT=va_h(h, j),
                                rhs=exs_[j][:, tq * 512:(tq + 1) * 512],
                                start=(j == 0 and not hastail), stop=last)

                    # packed tail FIRST so it flows through the normal
                    # scores->exp->PV software pipeline: head-even lanes at
                    # partitions 0:64, head-odd at 64:128; the 4 score MMs
                    # land on 4 distinct PE quadrants and run concurrently,
                    # one exp covers both heads, and the tail PVs open the
                    # po accumulations (start=True).
                    ext = None
                    if hastail:
                        ktA, ktB, kc0 = kt_pair(pr)
                        ps = pspool.tile([P, THALF], F32, name="ps",
                                         tag="ps", bufs=2)
                        for hh, rbase in ((h0, 0), (h1, 64)):
                            for tq in range(2):
                                if tq == 0:
                                    kt, kr = ktA, (hh % 2) * 64
                                else:
                                    kt, kr = ktB, (1 - hh % 2) * 64
                                qt, qr = qT_h(hh, tq)
                                nc.tensor.matmul(
                                    ps[rbase:rbase + 64,
                                       tq * 512:(tq + 1) * 512],
                                    lhsT=kt[kr:kr + 64, kc0:kc0 + 64],
                                    rhs=qt[qr:qr + 64, t0 + tq * 512:
                                           t0 + (tq + 1) * 512],
                                    start=True, stop=True)
                        ext = epool.tile([P, THALF], F16, name="ex",
                                         tag="ex", bufs=6)
                        nc.scalar.activation(
                            ext[:], ps[:], mybir.ActivationFunctionType.Exp,
                            bias=0.0, scale=SCALE)

                    def tail_pv(po_, rbase, vc, last):
                        for tq in range(2):
                            nc.tensor.matmul(
                                po_[0:65, tq * 512:(tq + 1) * 512],
                                lhsT=vat_pair(pr)[0][rbase:rbase + 64,
                                                     vc:vc + 65],
                                rhs=ext[rbase:rbase + 64,
                                        tq * 512:(tq + 1) * 512],
                                start=True, stop=last)

                    def full_head(h, po_, first):
                        exs = []
                        for j in range(nf):
                            ps = pspool.tile([P, THALF], F32, name="ps",
                                             tag="ps", bufs=2)
                            for tq in range(2):
                                kt, kr = kT_h(h, tq)
                                qt, qr = qT_h(h, tq)
                                nc.tensor.matmul(
                                    ps[:, tq * 512:(tq + 1) * 512],
                                    lhsT=kt[kr:kr + 64, j * P:(j + 1) * P],
                                    rhs=qt[qr:qr + 64, t0 + tq * 512:
                                           t0 + (tq + 1) * 512],
                                    start=True, stop=True)
                            ex = epool.tile([P, THALF], F16, name="ex",
                                            tag="ex", bufs=6)
                            nc.scalar.activation(
                                ex[:], ps[:],
                                mybir.ActivationFunctionType.Exp,
                                bias=0.0, scale=SCALE)
                            exs.append(ex)
                            if j == 0 and hastail:
                                # lag-1 slot: emit the pair's tail PVs here
                                vc0 = vat_pair(pr)[1]
                                if first:
                                    tail_pv(po0, 0, vc0, last=False)
                                    tail_pv(po1, 64, vc0 + 65, last=False)
                            if j >= 1:
                                pv(h, po_, exs, j - 1, last=False)
                            # inject t-half-0 projection chunks into the
                            # long heads of t-half 1; j=2/6 are far enough
                            # apart for the ACT backlog to recover
                            if th == 1 and h >= 4 and j in (2, 6):
                                proj_chunk(2 * (h - 4) + (0 if j == 2 else 1))
                        if nf >= 1:
                            pv(h, po_, exs, nf - 1, last=True)

                    full_head(h0, po0, first=True)
                    full_head(h1, po1, first=False)
                    if hastail and nf == 0:
                        vc0 = vat_pair(pr)[1]
                        tail_pv(po0, 0, vc0, last=True)
                        tail_pv(po1, 64, vc0 + 65, last=True)

                    # bridge the final normalization chain with dummy
                    # matmuls so the PE stays warm into the projection tail
                    if th == 1 and pr == 3:
                        norm(h0, po0, t0)
                        norm(h1, po1, t0)
                        warm_burst(14)
                    else:
                        norm(h0, po0, t0)
                        norm(h1, po1, t0)
            for tq in range(8, 16):
                proj_chunk(tq)

    nc.compile()
    return nc


_PROGRAMS = {}


def _get_program(key):
    if key not in _PROGRAMS:
        _PROGRAMS[key] = build_program(*key)
    return _PROGRAMS[key]


def _swap_halves(m):
    # [128k, N] -> swap the two 64-row halves within each 128-row block
    blocks = [m[i:i + P] for i in range(0, m.shape[0], P)]
    return np.vstack([np.vstack([b[64:P], b[0:64]]) for b in blocks])


def _prep_core_inputs(query, key, value, wT, pb, keeps, cfg):
    nf4, nf2, nf1, t4, t2, t1 = cfg
    NF = {4: max(nf4, 1), 2: max(nf2, 1), 1: max(nf1, 1)}
    NFR = {4: nf4, 2: nf2, 1: nf1}
    ins = []
    for b in range(B):
        qb = np.ascontiguousarray(query[:, b, :].T).astype(np.float16)
        qbd = np.vstack([qb, _swap_halves(qb)])

        def build_k(sub, idx, c0, c1, ks):
            z = np.zeros((P, NF[ks] * P), dtype=np.float16)
            g = sub[idx[:NFR[ks] * P]]
            n = g.shape[0]
            z[:, 0:n] = g[:, c0:c1].T.astype(np.float16)
            return np.vstack([z, _swap_halves(z)])

        def build_va(sub, idx, heads, W, ks):
            g = sub[idx[:NFR[ks] * P]]
            z = np.zeros((P, NF[ks] * W), dtype=np.float16)
            for j in range(NF[ks]):
                seg = g[j * P:(j + 1) * P]
                m = seg.shape[0]
                if m == 0:
                    break
                for i, h in enumerate(heads):
                    z[0:m, j * W + i * 65: j * W + i * 65 + 64] = \
                        seg[:, h * 64:(h + 1) * 64].astype(np.float16)
                    z[0:m, j * W + i * 65 + 64] = 1.0
            return z

        def build_kt(sub, idx, pairs, ks):
            # [2P, 64*len(pairs)]: per pair, head-even dims on rows 0:64,
            # head-odd on 64:128, tail lanes as columns; plus swapped copy
            z = np.zeros((P, 64 * len(pairs)), dtype=np.float16)
            g = sub[idx[NFR[ks] * P:]]
            n = g.shape[0]
            for i, (ha, hb) in enumerate(pairs):
                if n:
                    z[0:64, i * 64:i * 64 + n] = \
                        g[:, ha * 64:(ha + 1) * 64].T.astype(np.float16)
                    z[64:P, i * 64:i * 64 + n] = \
                        g[:, hb * 64:(hb + 1) * 64].T.astype(np.float16)
            return np.vstack([z, _swap_halves(z)])

        def build_vat(sub, idx, pairs, ks):
            z = np.zeros((P, 130 * len(pairs)), dtype=np.float16)
            g = sub[idx[NFR[ks] * P:]]
            n = g.shape[0]
            for i, (ha, hb) in enumerate(pairs):
                if n:
                    z[0:n, i * 130:i * 130 + 64] = \
                        g[:, ha * 64:(ha + 1) * 64].astype(np.float16)
                    z[0:n, i * 130 + 64] = 1.0
                    z[64:64 + n, i * 130 + 65:i * 130 + 129] = \
                        g[:, hb * 64:(hb + 1) * 64].astype(np.float16)
                    z[64:64 + n, i * 130 + 129] = 1.0
            return z

        kb, vb = key[:, b, :], value[:, b, :]
        i4, i2, i1 = keeps[4][b], keeps[2][b], keeps[1][b]
        ins.append({
            "qT": qbd,
            "k4T": build_k(kb[::4], i4, 0, 128, 4),
            "k2T": build_k(kb[::2], i2, 128, 256, 2),
            "k1Ta": build_k(kb, i1, 256, 384, 1),
            "k1Tb": build_k(kb, i1, 384, 512, 1),
            "va4": build_va(vb[::4], i4, [0, 1], 130, 4),
            "va2": build_va(vb[::2], i2, [2, 3], 130, 2),
            "va1": build_va(vb, i1, [4, 5, 6, 7], 260, 1),
            "kt4": build_kt(kb[::4], i4, [(0, 1)], 4),
            "kt2": build_kt(kb[::2], i2, [(2, 3)], 2),
            "kt1": build_kt(kb, i1, [(4, 5), (6, 7)], 1),
            "vat4": build_vat(vb[::4], i4, [(0, 1)], 4),
            "vat2": build_vat(vb[::2], i2, [(2, 3)], 2),
            "vat1": build_vat(vb, i1, [(4, 5), (6, 7)], 1),
            "wT": wT, "pb": pb,
        })
    return ins


def kernel(query, key, value, attn_mask, proj_w, proj_b, _trace=False,
           **run_kwargs):
    query = np.asarray(query, dtype=np.float32)
    key = np.asarray(key, dtype=np.float32)
    value = np.asarray(value, dtype=np.float32)
    mask = np.asarray(attn_mask).astype(bool)
    wT = np.ascontiguousarray(
        np.asarray(proj_w, dtype=np.float32).T).astype(np.float16)
    pb = np.ascontiguousarray(
        np.asarray(proj_b, dtype=np.float32).reshape(1, E))

    keeps = {ks: [np.flatnonzero(~mask[b, ::ks]) for b in range(B)]
             for ks in (4, 2, 1)}
    # Packed sub-128 tails (see build_program) measured SLOWER end-to-end
    # than padding up to full chunks (165us vs 171us median): the tail step
    # disrupts the scores/exp/PV software pipeline more than its saved
    # ACTIVATE instructions are worth. Keep the machinery but default to
    # padded full chunks.
    import os
    notails = os.environ.get("KERNEL_TAILS") != "1"
    cfg = []
    for ks in (4, 2, 1):
        mx = max(len(keeps[ks][b]) for b in range(B))
        nf, lt = divmod(mx, P)
        if notails and lt:
            nf, lt = nf + 1, 0
        if lt > 64:
            # tail too wide to pack two heads side by side: pad to a full
            # chunk instead
            nf, lt = nf + 1, 0
        cfg.append((nf, lt > 0))
    cfg = (cfg[0][0], cfg[1][0], cfg[2][0], cfg[0][1], cfg[1][1], cfg[2][1])

    nc = _get_program(cfg)
    ins = _prep_core_inputs(query, key, value, wT, pb, keeps, cfg)
    res = run_bass_kernel_spmd(nc, ins, list(range(B)), trace=_trace,
                               **run_kwargs)
    outs = [np.asarray(res.results[b]["out"]) for b in range(B)]
    full = np.concatenate(outs, axis=0)          # (B*T, E), b-major rows
    result = full.reshape(T, B, E)
    if _trace:
        return result, res
    return result
